# revision 18
# baseline (speedup 1.0000x reference)
"""Trainium2 Bass kernel for nn_Decoder (per-depth label classifier).

Math (per depth d with c_d labels, COUNTS=[16,128,512]):
    g_d = label_aware_embedding[:, idx_d, :].reshape(B, c_d*H)
    x_d = g_d @ W1_d.T                     # [B, H]
    logits_d = x_d @ Wp_d.T + bp_d         # [B, c_d]
    pred[:, idx_d] = logits_d

Sharding: the W1_d contraction dim (c_d*H) is split across 8 cores
(each core gets c_d/8 labels' worth of W1 columns plus the matching
gathered-embedding slice) and each core computes a partial x_d.
Because the predictor is linear in x, the cross-core reduction commutes
past it:  pred = (sum_i x_i) @ Wp.T = sum_i (x_i @ Wp.T).  So each core
runs the (tiny) predictor on its own partial x and the host unshard step
sums the 8 partial outputs and adds the bias once — no on-device
collective at all.

Device layout notes:
  - host pre-transposes so the contraction dim is the partition dim and
    every DMA reads a per-partition-contiguous span:
      w1t: [128, 328*512] bf16   ([p, k*512+n] = W1slice.T[k*128+p, n])
      gt:  [128, 328*64]  bf16   (same for g.T)
  - main matmul: lhsT = gt chunk [128,64] (stationary), rhs = w1t chunk
    [128,512] (moving) -> psum [64,512] accumulated per depth.
  - the predictor needs x.T; partial x is cast to bf16 and transposed on
    the PE via identity matmuls.
"""

import sys

sys.path.insert(0, "/opt/trn_rl_repo")

import numpy as np
import ml_dtypes

import concourse.bass as bass
import concourse.bacc as bacc
import concourse.tile as tile
import concourse.mybir as mybir
from concourse import bass_utils

BF16 = np.dtype(ml_dtypes.bfloat16)

N_CORES = 8
H = 512
B = 64
COUNTS = [16, 128, 512]
L = sum(COUNTS)  # 656

# Fixed label->depth assignment (identical to the reference's module-level rng)
_depths = np.random.default_rng(0).permutation(np.repeat(np.arange(1, 4), COUNTS))
IDX = [np.where(_depths == d)[0] for d in (1, 2, 3)]
ORDER = np.concatenate(IDX)

PER_CORE = [c // N_CORES for c in COUNTS]  # labels per core per depth: [2, 16, 64]
KCH = [n * H // 128 for n in PER_CORE]  # K-chunks per depth per core: [8, 64, 256]
NCH = sum(KCH)  # 328

# DMA group sizes (in K-chunks) per depth; small leading groups so the PE
# starts working as soon as possible, and small groups throughout so the
# warm PE never idles long enough (>3.4us) for the HAM clock gate to
# re-throttle it.
GROUPS = [[2, 6], [8] * 8, [8] * 32]

LABEL_OFF = [0, COUNTS[0], COUNTS[0] + COUNTS[1]]  # predT row offset per depth

_CACHE = {}


def _build_module():
    f32 = mybir.dt.float32
    bf16 = mybir.dt.bfloat16

    nc = bacc.Bacc("TRN2", target_bir_lowering=False, debug=False, num_devices=N_CORES)

    w1t = nc.dram_tensor("w1t", [128, NCH * H], bf16, kind="ExternalInput").ap()
    gt = nc.dram_tensor("gt", [128, NCH * B], bf16, kind="ExternalInput").ap()
    wpt = nc.dram_tensor("wpt", [128, 4 * L], bf16, kind="ExternalInput").ap()
    ident = nc.dram_tensor("ident", [128, 128], bf16, kind="ExternalInput").ap()
    predT = nc.dram_tensor("predT", [L, B], f32, kind="ExternalOutput").ap()

    with tile.TileContext(nc) as tc:
        with (
            tc.tile_pool(name="wpool", bufs=14) as wpool,
            tc.tile_pool(name="gpool", bufs=14) as gpool,
            tc.tile_pool(name="consts", bufs=1) as consts,
            tc.tile_pool(name="xpool", bufs=1) as xpool,
            tc.tile_pool(name="spool", bufs=6) as spool,
            tc.tile_pool(name="ps_x", bufs=3, space="PSUM") as ps_x,
            tc.tile_pool(name="ps_t", bufs=2, space="PSUM") as ps_t,
            tc.tile_pool(name="ps_p", bufs=2, space="PSUM") as ps_p,
        ):
            # constants go on the gpsimd (SWDGE) queue so they don't delay
            # the first weight/activation loads on the HWDGE rings
            wpt_sb = consts.tile([128, 4 * L], bf16)
            nc.gpsimd.dma_start(wpt_sb[:], wpt[:])
            id_sb = consts.tile([128, 128], bf16)
            nc.gpsimd.dma_start(id_sb[:], ident[:])

            # ---- main matmuls: partial x_d = g_d @ W1_d.T, all 3 depths
            # back-to-back so the PE instruction stream has no mid-stream
            # dependencies on other engines (PE executes in order) ----
            xb_tiles = []
            chunk_off = 0
            for d in range(3):
                nch = KCH[d]
                ps = ps_x.tile([B, H], f32, name=f"psx{d}", tag="psx")
                g0 = 0
                for gi, gl in enumerate(GROUPS[d]):
                    c0 = chunk_off + g0
                    # alternate the two HWDGE rings so the SDMA engines always
                    # have the next group's descriptors queued
                    ring_w = nc.sync if gi % 2 == 0 else nc.scalar
                    ring_g = nc.scalar if gi % 2 == 0 else nc.sync
                    wt = wpool.tile([128, gl * H], bf16, name="wt", tag="w")
                    ring_w.dma_start(wt[:], w1t[:, c0 * H : (c0 + gl) * H])
                    gtt = gpool.tile([128, gl * B], bf16, name="gtt", tag="g")
                    ring_g.dma_start(gtt[:], gt[:, c0 * B : (c0 + gl) * B])
                    for j in range(gl):
                        nc.tensor.matmul(
                            ps[:],
                            lhsT=gtt[:, j * B : (j + 1) * B],
                            rhs=wt[:, j * H : (j + 1) * H],
                            start=(g0 + j == 0),
                            stop=(g0 + j == nch - 1),
                        )
                    g0 += gl
                chunk_off += nch
                # cast partial x to bf16 early (DVE runs concurrently with
                # the next depth's matmuls)
                xb = xpool.tile([B, H], bf16, name=f"xb{d}", tag=f"xb{d}")
                nc.vector.tensor_copy(xb[:], ps[:])
                xb_tiles.append(xb)

            # ---- tail: transpose partial x on the PE, then the partial
            # predictor logits_d.T = Wp_d @ x_d.T ----
            for d in range(3):
                xb = xb_tiles[d]
                pt = ps_t.tile([128, 4 * B], bf16, name=f"pt{d}", tag="pt")
                for k in range(4):
                    nc.tensor.transpose(
                        pt[:, k * B : (k + 1) * B],
                        xb[:, k * 128 : (k + 1) * 128],
                        id_sb[:B, :B],
                    )
                xT = xpool.tile([128, 4 * B], bf16, name=f"xT{d}", tag=f"xT{d}")
                nc.vector.tensor_copy(xT[:], pt[:])

                c = COUNTS[d]
                nm = (c + 127) // 128
                pp = ps_p.tile([128, nm * B], f32, name=f"pp{d}", tag="pp")
                for m in range(nm):
                    ms = min(128, c - m * 128)
                    for k in range(4):
                        nc.tensor.matmul(
                            pp[:ms, m * B : m * B + B],
                            lhsT=wpt_sb[
                                :, k * L + LABEL_OFF[d] + m * 128 : k * L
                                + LABEL_OFF[d] + m * 128 + ms
                            ],
                            rhs=xT[:, k * B : (k + 1) * B],
                            start=(k == 0),
                            stop=(k == 3),
                        )
                for m in range(nm):
                    ms = min(128, c - m * 128)
                    po = spool.tile([128, B], f32, name=f"po{d}_{m}", tag="po")
                    nc.vector.tensor_copy(po[:ms, :], pp[:ms, m * B : m * B + B])
                    row0 = LABEL_OFF[d] + m * 128
                    nc.sync.dma_start(predT[row0 : row0 + ms, :], po[:ms, :])

    nc.finalize()
    return nc


def _prep_inputs(inputs):
    emb = np.asarray(inputs["label_aware_embedding"])
    W1s = [np.asarray(inputs[f"W1_{i + 1}"]) for i in range(3)]
    Wps = [np.asarray(inputs[f"Wp_{i + 1}"]) for i in range(3)]

    emb_bf = emb.astype(BF16)

    w1t_all = np.empty((N_CORES, 128, NCH * H), BF16)
    gt_all = np.empty((N_CORES, 128, NCH * B), BF16)
    off = 0
    for d in range(3):
        ch = KCH[d]
        W1T = np.ascontiguousarray(W1s[d].astype(BF16).T)  # [c*H, 512]
        w1t_all[:, :, off * H : (off + ch) * H] = (
            W1T.reshape(N_CORES, ch, 128, H)
            .transpose(0, 2, 1, 3)
            .reshape(N_CORES, 128, ch * H)
        )
        ge = emb_bf[:, IDX[d], :]  # [B, c, H]
        GT = ge.transpose(1, 2, 0).reshape(-1, B)  # [c*H, 64]
        gt_all[:, :, off * B : (off + ch) * B] = (
            GT.reshape(N_CORES, ch, 128, B)
            .transpose(0, 2, 1, 3)
            .reshape(N_CORES, 128, ch * B)
        )
        off += ch

    WPT = np.concatenate([Wp.T for Wp in Wps], axis=1).astype(BF16)  # [512, 656]
    wpt_pack = np.ascontiguousarray(
        WPT.reshape(4, 128, L).transpose(1, 0, 2).reshape(128, 4 * L)
    )

    ident = np.eye(128, dtype=BF16)

    in_maps = []
    for c in range(N_CORES):
        in_maps.append(
            {
                "w1t": w1t_all[c],
                "gt": gt_all[c],
                "wpt": wpt_pack,
                "ident": ident,
            }
        )
    return in_maps


LAST_RESULTS = None


def kernel(**inputs):
    global LAST_RESULTS
    if "nc" not in _CACHE:
        _CACHE["nc"] = _build_module()
    nc = _CACHE["nc"]
    in_maps = _prep_inputs(inputs)
    try:
        res = bass_utils.run_bass_kernel_spmd(
            nc, in_maps, core_ids=list(range(N_CORES))
        )
    except Exception:
        # transient NRT device errors have been observed; retry once
        res = bass_utils.run_bass_kernel_spmd(
            nc, in_maps, core_ids=list(range(N_CORES))
        )
    LAST_RESULTS = res

    # unshard: contraction was sharded, so the full predictor output is the
    # sum of the per-core partials; add the bias once at the end.
    total = np.zeros((L, B), np.float64)
    for c in range(N_CORES):
        total += res.results[c]["predT"]
    bias = np.concatenate([np.asarray(inputs[f"bp_{i + 1}"]) for i in range(3)])
    total += bias.astype(np.float64)[:, None]
    out = np.empty((B, L), np.float32)
    out[:, ORDER] = total.T.astype(np.float32)
    return out


# revision 19
# speedup vs baseline: 1.0024x; 1.0024x over previous
"""Trainium2 Bass kernel for nn_Decoder (per-depth label classifier).

Math (per depth d with c_d labels, COUNTS=[16,128,512]):
    g_d = label_aware_embedding[:, idx_d, :].reshape(B, c_d*H)
    x_d = g_d @ W1_d.T                     # [B, H]
    logits_d = x_d @ Wp_d.T + bp_d         # [B, c_d]
    pred[:, idx_d] = logits_d

Sharding: the W1_d contraction dim (c_d*H) is split across 8 cores
(each core gets c_d/8 labels' worth of W1 columns plus the matching
gathered-embedding slice) and each core computes a partial x_d.
Because the predictor is linear in x, the cross-core reduction commutes
past it:  pred = (sum_i x_i) @ Wp.T = sum_i (x_i @ Wp.T).  So each core
runs the (tiny) predictor on its own partial x and the host unshard step
sums the 8 partial outputs and adds the bias once — no on-device
collective at all.

Device layout notes:
  - host pre-transposes so the contraction dim is the partition dim and
    every DMA reads a per-partition-contiguous span:
      w1t: [128, 328*512] bf16   ([p, k*512+n] = W1slice.T[k*128+p, n])
      gt:  [128, 328*64]  bf16   (same for g.T)
  - main matmul: lhsT = gt chunk [128,64] (stationary), rhs = w1t chunk
    [128,512] (moving) -> psum [64,512] accumulated per depth.
  - the predictor needs x.T; partial x is cast to bf16 and transposed on
    the PE via identity matmuls.
"""

import sys

sys.path.insert(0, "/opt/trn_rl_repo")

import numpy as np
import ml_dtypes

import concourse.bass as bass
import concourse.bacc as bacc
import concourse.tile as tile
import concourse.mybir as mybir
from concourse import bass_utils

BF16 = np.dtype(ml_dtypes.bfloat16)

N_CORES = 8
H = 512
B = 64
COUNTS = [16, 128, 512]
L = sum(COUNTS)  # 656

# Fixed label->depth assignment (identical to the reference's module-level rng)
_depths = np.random.default_rng(0).permutation(np.repeat(np.arange(1, 4), COUNTS))
IDX = [np.where(_depths == d)[0] for d in (1, 2, 3)]
ORDER = np.concatenate(IDX)

PER_CORE = [c // N_CORES for c in COUNTS]  # labels per core per depth: [2, 16, 64]
KCH = [n * H // 128 for n in PER_CORE]  # K-chunks per depth per core: [8, 64, 256]
NCH = sum(KCH)  # 328

# DMA group sizes (in K-chunks) per depth; small leading groups so the PE
# starts working as soon as possible, and small groups throughout so the
# warm PE never idles long enough (>3.4us) for the HAM clock gate to
# re-throttle it.
GROUPS = [[2, 6], [8] * 8, [8] * 32]

LABEL_OFF = [0, COUNTS[0], COUNTS[0] + COUNTS[1]]  # predT row offset per depth

_CACHE = {}


def _build_module():
    f32 = mybir.dt.float32
    bf16 = mybir.dt.bfloat16

    nc = bacc.Bacc("TRN2", target_bir_lowering=False, debug=False, num_devices=N_CORES)

    w1t = nc.dram_tensor("w1t", [128, NCH * H], bf16, kind="ExternalInput").ap()
    gt = nc.dram_tensor("gt", [128, NCH * B], bf16, kind="ExternalInput").ap()
    wpt = nc.dram_tensor("wpt", [128, 4 * L], bf16, kind="ExternalInput").ap()
    ident = nc.dram_tensor("ident", [128, 128], bf16, kind="ExternalInput").ap()
    predT = nc.dram_tensor("predT", [L, B], f32, kind="ExternalOutput").ap()

    with tile.TileContext(nc) as tc:
        with (
            tc.tile_pool(name="wpool", bufs=10) as wpool,
            tc.tile_pool(name="gpool", bufs=10) as gpool,
            tc.tile_pool(name="consts", bufs=1) as consts,
            tc.tile_pool(name="xpool", bufs=1) as xpool,
            tc.tile_pool(name="spool", bufs=6) as spool,
            tc.tile_pool(name="ps_x", bufs=3, space="PSUM") as ps_x,
            tc.tile_pool(name="ps_t", bufs=2, space="PSUM") as ps_t,
            tc.tile_pool(name="ps_p", bufs=2, space="PSUM") as ps_p,
        ):
            # constants go on the gpsimd (SWDGE) queue so they don't delay
            # the first weight/activation loads on the HWDGE rings
            wpt_sb = consts.tile([128, 4 * L], bf16)
            nc.gpsimd.dma_start(wpt_sb[:], wpt[:])
            id_sb = consts.tile([128, 128], bf16)
            nc.gpsimd.dma_start(id_sb[:], ident[:])

            # ---- main matmuls: partial x_d = g_d @ W1_d.T, all 3 depths
            # back-to-back so the PE instruction stream has no mid-stream
            # dependencies on other engines (PE executes in order) ----
            xb_tiles = []
            chunk_off = 0
            for d in range(3):
                nch = KCH[d]
                ps = ps_x.tile([B, H], f32, name=f"psx{d}", tag="psx")
                g0 = 0
                for gi, gl in enumerate(GROUPS[d]):
                    c0 = chunk_off + g0
                    # alternate the two HWDGE rings so the SDMA engines always
                    # have the next group's descriptors queued
                    ring_w = nc.sync if gi % 2 == 0 else nc.scalar
                    ring_g = nc.scalar if gi % 2 == 0 else nc.sync
                    wt = wpool.tile([128, gl * H], bf16, name="wt", tag="w")
                    ring_w.dma_start(wt[:], w1t[:, c0 * H : (c0 + gl) * H])
                    gtt = gpool.tile([128, gl * B], bf16, name="gtt", tag="g")
                    ring_g.dma_start(gtt[:], gt[:, c0 * B : (c0 + gl) * B])
                    for j in range(gl):
                        nc.tensor.matmul(
                            ps[:],
                            lhsT=gtt[:, j * B : (j + 1) * B],
                            rhs=wt[:, j * H : (j + 1) * H],
                            start=(g0 + j == 0),
                            stop=(g0 + j == nch - 1),
                        )
                    g0 += gl
                chunk_off += nch
                # cast partial x to bf16 early (DVE runs concurrently with
                # the next depth's matmuls)
                xb = xpool.tile([B, H], bf16, name=f"xb{d}", tag=f"xb{d}")
                nc.vector.tensor_copy(xb[:], ps[:])
                xb_tiles.append(xb)

            # ---- tail: transpose partial x on the PE, then the partial
            # predictor logits_d.T = Wp_d @ x_d.T ----
            for d in range(3):
                xb = xb_tiles[d]
                pt = ps_t.tile([128, 4 * B], bf16, name=f"pt{d}", tag="pt")
                for k in range(4):
                    nc.tensor.transpose(
                        pt[:, k * B : (k + 1) * B],
                        xb[:, k * 128 : (k + 1) * 128],
                        id_sb[:B, :B],
                    )
                xT = xpool.tile([128, 4 * B], bf16, name=f"xT{d}", tag=f"xT{d}")
                nc.vector.tensor_copy(xT[:], pt[:])

                c = COUNTS[d]
                nm = (c + 127) // 128
                pp = ps_p.tile([128, nm * B], f32, name=f"pp{d}", tag="pp")
                for m in range(nm):
                    ms = min(128, c - m * 128)
                    for k in range(4):
                        nc.tensor.matmul(
                            pp[:ms, m * B : m * B + B],
                            lhsT=wpt_sb[
                                :, k * L + LABEL_OFF[d] + m * 128 : k * L
                                + LABEL_OFF[d] + m * 128 + ms
                            ],
                            rhs=xT[:, k * B : (k + 1) * B],
                            start=(k == 0),
                            stop=(k == 3),
                        )
                for m in range(nm):
                    ms = min(128, c - m * 128)
                    po = spool.tile([128, B], f32, name=f"po{d}_{m}", tag="po")
                    nc.vector.tensor_copy(po[:ms, :], pp[:ms, m * B : m * B + B])
                    row0 = LABEL_OFF[d] + m * 128
                    nc.sync.dma_start(predT[row0 : row0 + ms, :], po[:ms, :])

    nc.finalize()
    return nc


def _prep_inputs(inputs):
    emb = np.asarray(inputs["label_aware_embedding"])
    W1s = [np.asarray(inputs[f"W1_{i + 1}"]) for i in range(3)]
    Wps = [np.asarray(inputs[f"Wp_{i + 1}"]) for i in range(3)]

    emb_bf = emb.astype(BF16)

    w1t_all = np.empty((N_CORES, 128, NCH * H), BF16)
    gt_all = np.empty((N_CORES, 128, NCH * B), BF16)
    off = 0
    for d in range(3):
        ch = KCH[d]
        W1T = np.ascontiguousarray(W1s[d].astype(BF16).T)  # [c*H, 512]
        w1t_all[:, :, off * H : (off + ch) * H] = (
            W1T.reshape(N_CORES, ch, 128, H)
            .transpose(0, 2, 1, 3)
            .reshape(N_CORES, 128, ch * H)
        )
        ge = emb_bf[:, IDX[d], :]  # [B, c, H]
        GT = ge.transpose(1, 2, 0).reshape(-1, B)  # [c*H, 64]
        gt_all[:, :, off * B : (off + ch) * B] = (
            GT.reshape(N_CORES, ch, 128, B)
            .transpose(0, 2, 1, 3)
            .reshape(N_CORES, 128, ch * B)
        )
        off += ch

    WPT = np.concatenate([Wp.T for Wp in Wps], axis=1).astype(BF16)  # [512, 656]
    wpt_pack = np.ascontiguousarray(
        WPT.reshape(4, 128, L).transpose(1, 0, 2).reshape(128, 4 * L)
    )

    ident = np.eye(128, dtype=BF16)

    in_maps = []
    for c in range(N_CORES):
        in_maps.append(
            {
                "w1t": w1t_all[c],
                "gt": gt_all[c],
                "wpt": wpt_pack,
                "ident": ident,
            }
        )
    return in_maps


LAST_RESULTS = None


def kernel(**inputs):
    global LAST_RESULTS
    if "nc" not in _CACHE:
        _CACHE["nc"] = _build_module()
    nc = _CACHE["nc"]
    in_maps = _prep_inputs(inputs)
    try:
        res = bass_utils.run_bass_kernel_spmd(
            nc, in_maps, core_ids=list(range(N_CORES))
        )
    except Exception:
        # transient NRT device errors have been observed; retry once
        res = bass_utils.run_bass_kernel_spmd(
            nc, in_maps, core_ids=list(range(N_CORES))
        )
    LAST_RESULTS = res

    # unshard: contraction was sharded, so the full predictor output is the
    # sum of the per-core partials; add the bias once at the end.
    total = np.zeros((L, B), np.float64)
    for c in range(N_CORES):
        total += res.results[c]["predT"]
    bias = np.concatenate([np.asarray(inputs[f"bp_{i + 1}"]) for i in range(3)])
    total += bias.astype(np.float64)[:, None]
    out = np.empty((B, L), np.float32)
    out[:, ORDER] = total.T.astype(np.float32)
    return out


# revision 21
# speedup vs baseline: 1.0875x; 1.0849x over previous
"""Trainium2 Bass kernel for nn_Decoder (per-depth label classifier).

Math (per depth d with c_d labels, COUNTS=[16,128,512]):
    g_d = label_aware_embedding[:, idx_d, :].reshape(B, c_d*H)
    x_d = g_d @ W1_d.T                     # [B, H]
    logits_d = x_d @ Wp_d.T + bp_d         # [B, c_d]
    pred[:, idx_d] = logits_d

Sharding: the W1_d contraction dim (c_d*H) is split across 8 cores
(each core gets c_d/8 labels' worth of W1 columns plus the matching
gathered-embedding slice) and each core computes a partial x_d.
Because the predictor is linear in x, the cross-core reduction commutes
past it:  pred = (sum_i x_i) @ Wp.T = sum_i (x_i @ Wp.T).  So each core
runs the (tiny) predictor on its own partial x and the host unshard step
sums the 8 partial outputs and adds the bias once — no on-device
collective at all.

Device layout notes:
  - host pre-transposes so the contraction dim is the partition dim and
    every DMA reads a per-partition-contiguous span:
      w1t: [128, 328*512] bf16   ([p, k*512+n] = W1slice.T[k*128+p, n])
      gt:  [128, 328*64]  bf16   (same for g.T)
  - main matmul: lhsT = gt chunk [128,64] (stationary), rhs = w1t chunk
    [128,512] (moving) -> psum [64,512] accumulated per depth.
  - the predictor needs x.T; partial x is cast to bf16 and transposed on
    the PE via identity matmuls.
"""

import sys

sys.path.insert(0, "/opt/trn_rl_repo")

import numpy as np
import ml_dtypes

import concourse.bass as bass
import concourse.bacc as bacc
import concourse.tile as tile
import concourse.mybir as mybir
from concourse import bass_utils

BF16 = np.dtype(ml_dtypes.bfloat16)

N_CORES = 8
H = 512
B = 64
COUNTS = [16, 128, 512]
L = sum(COUNTS)  # 656

# Fixed label->depth assignment (identical to the reference's module-level rng)
_depths = np.random.default_rng(0).permutation(np.repeat(np.arange(1, 4), COUNTS))
IDX = [np.where(_depths == d)[0] for d in (1, 2, 3)]
ORDER = np.concatenate(IDX)

PER_CORE = [c // N_CORES for c in COUNTS]  # labels per core per depth: [2, 16, 64]
KCH = [n * H // 128 for n in PER_CORE]  # K-chunks per depth per core: [8, 64, 256]
NCH = sum(KCH)  # 328

# DMA group sizes (in K-chunks) per depth; small leading groups so the PE
# starts working as soon as possible, and small groups throughout so the
# warm PE never idles long enough (>3.4us) for the HAM clock gate to
# re-throttle it.
GROUPS = [[2, 6], [8] * 8, [8] * 32]

LABEL_OFF = [0, COUNTS[0], COUNTS[0] + COUNTS[1]]  # predT row offset per depth

_CACHE = {}


def _build_module():
    f32 = mybir.dt.float32
    bf16 = mybir.dt.bfloat16

    nc = bacc.Bacc("TRN2", target_bir_lowering=False, debug=False, num_devices=N_CORES)

    w1t = nc.dram_tensor("w1t", [128, NCH * H], bf16, kind="ExternalInput").ap()
    gt = nc.dram_tensor("gt", [128, NCH * B], bf16, kind="ExternalInput").ap()
    wpt = nc.dram_tensor("wpt", [128, 4 * L], bf16, kind="ExternalInput").ap()
    ident = nc.dram_tensor("ident", [128, 128], bf16, kind="ExternalInput").ap()
    predT = nc.dram_tensor("predT", [L, B], f32, kind="ExternalOutput").ap()

    with tile.TileContext(nc) as tc:
        with (
            tc.tile_pool(name="wpool", bufs=10) as wpool,
            tc.tile_pool(name="gpool", bufs=10) as gpool,
            tc.tile_pool(name="consts", bufs=1) as consts,
            tc.tile_pool(name="xpool", bufs=1) as xpool,
            tc.tile_pool(name="spool", bufs=6) as spool,
            tc.tile_pool(name="ps_x", bufs=3, space="PSUM") as ps_x,
            tc.tile_pool(name="ps_t", bufs=2, space="PSUM") as ps_t,
            tc.tile_pool(name="ps_p", bufs=2, space="PSUM") as ps_p,
        ):
            # constants go on the gpsimd (SWDGE) queue so they don't delay
            # the first weight/activation loads on the HWDGE rings
            wpt_sb = consts.tile([128, 4 * L], bf16)
            nc.gpsimd.dma_start(wpt_sb[:], wpt[:])
            id_sb = consts.tile([128, 128], bf16)
            nc.gpsimd.dma_start(id_sb[:], ident[:])

            # ---- main matmuls: partial x_d = g_d @ W1_d.T, all 3 depths
            # back-to-back so the PE instruction stream has no mid-stream
            # dependencies on other engines (PE executes in order) ----
            # ---- PE pre-warm: ~4us of dummy matmuls during the (otherwise
            # idle) DMA ramp so the HAM clock gate reaches K=8/8 before the
            # real matmul stream starts ----
            warm = ps_t.tile([128, H], f32, name="warm", tag="pt")
            for _ in range(10):
                nc.tensor.matmul(
                    warm[:, :],
                    lhsT=id_sb[:, :],
                    rhs=wpt_sb[:, :H],
                    start=True,
                    stop=True,
                )

            xb_tiles = []
            chunk_off = 0
            for d in range(3):
                nch = KCH[d]
                ps = ps_x.tile([B, H], f32, name=f"psx{d}", tag="psx")
                g0 = 0
                for gi, gl in enumerate(GROUPS[d]):
                    c0 = chunk_off + g0
                    # alternate the two HWDGE rings so the SDMA engines always
                    # have the next group's descriptors queued
                    ring_w = nc.sync if gi % 2 == 0 else nc.scalar
                    ring_g = nc.scalar if gi % 2 == 0 else nc.sync
                    wt = wpool.tile([128, gl * H], bf16, name="wt", tag="w")
                    ring_w.dma_start(wt[:], w1t[:, c0 * H : (c0 + gl) * H])
                    gtt = gpool.tile([128, gl * B], bf16, name="gtt", tag="g")
                    ring_g.dma_start(gtt[:], gt[:, c0 * B : (c0 + gl) * B])
                    for j in range(gl):
                        nc.tensor.matmul(
                            ps[:],
                            lhsT=gtt[:, j * B : (j + 1) * B],
                            rhs=wt[:, j * H : (j + 1) * H],
                            start=(g0 + j == 0),
                            stop=(g0 + j == nch - 1),
                        )
                    g0 += gl
                chunk_off += nch
                # cast partial x to bf16 early (DVE runs concurrently with
                # the next depth's matmuls)
                xb = xpool.tile([B, H], bf16, name=f"xb{d}", tag=f"xb{d}")
                nc.vector.tensor_copy(xb[:], ps[:])
                xb_tiles.append(xb)

            # ---- tail: transpose partial x on the PE, then the partial
            # predictor logits_d.T = Wp_d @ x_d.T ----
            for d in range(3):
                xb = xb_tiles[d]
                pt = ps_t.tile([128, 4 * B], bf16, name=f"pt{d}", tag="pt")
                for k in range(4):
                    nc.tensor.transpose(
                        pt[:, k * B : (k + 1) * B],
                        xb[:, k * 128 : (k + 1) * 128],
                        id_sb[:B, :B],
                    )
                xT = xpool.tile([128, 4 * B], bf16, name=f"xT{d}", tag=f"xT{d}")
                nc.vector.tensor_copy(xT[:], pt[:])

                c = COUNTS[d]
                nm = (c + 127) // 128
                pp = ps_p.tile([128, nm * B], f32, name=f"pp{d}", tag="pp")
                for m in range(nm):
                    ms = min(128, c - m * 128)
                    for k in range(4):
                        nc.tensor.matmul(
                            pp[:ms, m * B : m * B + B],
                            lhsT=wpt_sb[
                                :, k * L + LABEL_OFF[d] + m * 128 : k * L
                                + LABEL_OFF[d] + m * 128 + ms
                            ],
                            rhs=xT[:, k * B : (k + 1) * B],
                            start=(k == 0),
                            stop=(k == 3),
                        )
                for m in range(nm):
                    ms = min(128, c - m * 128)
                    po = spool.tile([128, B], f32, name=f"po{d}_{m}", tag="po")
                    nc.vector.tensor_copy(po[:ms, :], pp[:ms, m * B : m * B + B])
                    row0 = LABEL_OFF[d] + m * 128
                    nc.sync.dma_start(predT[row0 : row0 + ms, :], po[:ms, :])

    nc.finalize()
    return nc


def _prep_inputs(inputs):
    emb = np.asarray(inputs["label_aware_embedding"])
    W1s = [np.asarray(inputs[f"W1_{i + 1}"]) for i in range(3)]
    Wps = [np.asarray(inputs[f"Wp_{i + 1}"]) for i in range(3)]

    emb_bf = emb.astype(BF16)

    w1t_all = np.empty((N_CORES, 128, NCH * H), BF16)
    gt_all = np.empty((N_CORES, 128, NCH * B), BF16)
    off = 0
    for d in range(3):
        ch = KCH[d]
        W1T = np.ascontiguousarray(W1s[d].astype(BF16).T)  # [c*H, 512]
        w1t_all[:, :, off * H : (off + ch) * H] = (
            W1T.reshape(N_CORES, ch, 128, H)
            .transpose(0, 2, 1, 3)
            .reshape(N_CORES, 128, ch * H)
        )
        ge = emb_bf[:, IDX[d], :]  # [B, c, H]
        GT = ge.transpose(1, 2, 0).reshape(-1, B)  # [c*H, 64]
        gt_all[:, :, off * B : (off + ch) * B] = (
            GT.reshape(N_CORES, ch, 128, B)
            .transpose(0, 2, 1, 3)
            .reshape(N_CORES, 128, ch * B)
        )
        off += ch

    WPT = np.concatenate([Wp.T for Wp in Wps], axis=1).astype(BF16)  # [512, 656]
    wpt_pack = np.ascontiguousarray(
        WPT.reshape(4, 128, L).transpose(1, 0, 2).reshape(128, 4 * L)
    )

    ident = np.eye(128, dtype=BF16)

    in_maps = []
    for c in range(N_CORES):
        in_maps.append(
            {
                "w1t": w1t_all[c],
                "gt": gt_all[c],
                "wpt": wpt_pack,
                "ident": ident,
            }
        )
    return in_maps


LAST_RESULTS = None


def kernel(**inputs):
    global LAST_RESULTS
    if "nc" not in _CACHE:
        _CACHE["nc"] = _build_module()
    nc = _CACHE["nc"]
    in_maps = _prep_inputs(inputs)
    try:
        res = bass_utils.run_bass_kernel_spmd(
            nc, in_maps, core_ids=list(range(N_CORES))
        )
    except Exception:
        # transient NRT device errors have been observed; retry once
        res = bass_utils.run_bass_kernel_spmd(
            nc, in_maps, core_ids=list(range(N_CORES))
        )
    LAST_RESULTS = res

    # unshard: contraction was sharded, so the full predictor output is the
    # sum of the per-core partials; add the bias once at the end.
    total = np.zeros((L, B), np.float64)
    for c in range(N_CORES):
        total += res.results[c]["predT"]
    bias = np.concatenate([np.asarray(inputs[f"bp_{i + 1}"]) for i in range(3)])
    total += bias.astype(np.float64)[:, None]
    out = np.empty((B, L), np.float32)
    out[:, ORDER] = total.T.astype(np.float32)
    return out


# revision 31
# speedup vs baseline: 1.1874x; 1.0918x over previous
"""Trainium2 Bass kernel for nn_Decoder (per-depth label classifier).

Math (per depth d with c_d labels, COUNTS=[16,128,512]):
    g_d = label_aware_embedding[:, idx_d, :].reshape(B, c_d*H)
    x_d = g_d @ W1_d.T                     # [B, H]
    logits_d = x_d @ Wp_d.T + bp_d         # [B, c_d]
    pred[:, idx_d] = logits_d

Sharding: the W1_d contraction dim (c_d*H) is split across 8 cores
(each core gets c_d/8 labels' worth of W1 columns plus the matching
gathered-embedding slice) and each core computes a partial x_d.
Because the predictor is linear in x, the cross-core reduction commutes
past it:  pred = (sum_i x_i) @ Wp.T = sum_i (x_i @ Wp.T).  So each core
runs the (tiny) predictor on its own partial x and the host unshard step
sums the 8 partial outputs and adds the bias once — no on-device
collective at all.

Device layout notes:
  - host pre-transposes so the contraction dim is the partition dim and
    every DMA reads a per-partition-contiguous span:
      w1t: [128, 328*512] bf16   ([p, k*512+n] = W1slice.T[k*128+p, n])
      gt:  [128, 328*64]  bf16   (same for g.T)
  - main matmul: lhsT = gt chunk [128,64] (stationary), rhs = w1t chunk
    [128,512] (moving) -> psum [64,512] accumulated per depth.
  - the predictor needs x.T; partial x is cast to bf16 and transposed on
    the PE via identity matmuls.
"""

import sys

sys.path.insert(0, "/opt/trn_rl_repo")

import numpy as np
import ml_dtypes

import concourse.bass as bass
import concourse.bacc as bacc
import concourse.tile as tile
import concourse.mybir as mybir
from concourse import bass_utils

# bass_utils' trace path (taken when BASS_TRACE is set in the environment)
# imports antenv.axon_hooks, which this image's antenv package lacks.  Provide
# it: wire the real NTFF hook from trn_agent_boot when available, else a stub
# that degrades to an untraced run.  Also make the artifact upload a no-op
# (no bucket access here).
try:
    from antenv import axon_hooks as _axon_hooks  # noqa: F401
except ImportError:
    import types as _types

    def _make_hook():
        try:
            import trn_agent_boot.trn_boot as _tb

            return _tb._ntff_profile_via_ctypes("/opt/axon/libaxon_pjrt.so")
        except Exception:
            return None

    _hook = _make_hook()
    _mod = _types.ModuleType("antenv.axon_hooks")
    _mod.get_axon_ntff_profile_hook = lambda: _hook
    _mod.set_axon_ntff_profile_hook = lambda h: None
    sys.modules["antenv.axon_hooks"] = _mod
    bass_utils.upload_artifacts = lambda tmpdir: tmpdir

BF16 = np.dtype(ml_dtypes.bfloat16)

N_CORES = 8
H = 512
B = 64
COUNTS = [16, 128, 512]
L = sum(COUNTS)  # 656

# Fixed label->depth assignment (identical to the reference's module-level rng)
_depths = np.random.default_rng(0).permutation(np.repeat(np.arange(1, 4), COUNTS))
IDX = [np.where(_depths == d)[0] for d in (1, 2, 3)]
ORDER = np.concatenate(IDX)

PER_CORE = [c // N_CORES for c in COUNTS]  # labels per core per depth: [2, 16, 64]
KCH = [n * H // 128 for n in PER_CORE]  # K-chunks per depth per core: [8, 64, 256]
NCH = sum(KCH)  # 328

# DMA group sizes (in K-chunks) per depth; small leading groups so the PE
# starts working as soon as possible, and small groups throughout so the
# warm PE never idles long enough (>3.4us) for the HAM clock gate to
# re-throttle it.
GROUPS = [[2, 6], [8] * 8, [8] * 32]

LABEL_OFF = [0, COUNTS[0], COUNTS[0] + COUNTS[1]]  # predT row offset per depth

_CACHE = {}


def _build_module():
    f32 = mybir.dt.float32
    bf16 = mybir.dt.bfloat16

    nc = bacc.Bacc("TRN2", target_bir_lowering=False, debug=False, num_devices=N_CORES)

    WG = H + B  # 576: per K-chunk, 512 cols of W1.T then 64 cols of g.T
    wg = nc.dram_tensor("wg", [128, NCH * WG], bf16, kind="ExternalInput").ap()
    wpt = nc.dram_tensor("wpt", [128, 4 * L], bf16, kind="ExternalInput").ap()
    ident = nc.dram_tensor("ident", [128, 128], bf16, kind="ExternalInput").ap()
    predT = nc.dram_tensor("predT", [L, B], f32, kind="ExternalOutput").ap()

    with tile.TileContext(nc) as tc:
        with (
            tc.tile_pool(name="wpool", bufs=10) as wpool,
            tc.tile_pool(name="consts", bufs=1) as consts,
            tc.tile_pool(name="xpool", bufs=1) as xpool,
            tc.tile_pool(name="spool", bufs=6) as spool,
            tc.tile_pool(name="ps_x", bufs=3, space="PSUM") as ps_x,
            tc.tile_pool(name="ps_t", bufs=2, space="PSUM") as ps_t,
            tc.tile_pool(name="ps_p", bufs=2, space="PSUM") as ps_p,
        ):
            # constants go on the gpsimd (SWDGE) queue so they don't delay
            # the first weight/activation loads on the HWDGE rings
            wpt_sb = consts.tile([128, 4 * L], bf16)
            nc.gpsimd.dma_start(wpt_sb[:], wpt[:])
            id_sb = consts.tile([128, 128], bf16)
            nc.gpsimd.dma_start(id_sb[:], ident[:])

            # ---- main matmuls: partial x_d = g_d @ W1_d.T, all 3 depths
            # back-to-back so the PE instruction stream has no mid-stream
            # dependencies on other engines (PE executes in order) ----
            # depth-d tail: transpose partial x on the PE, then the partial
            # predictor logits_d.T = Wp_d @ x_d.T.  Emitted in the middle of
            # depth d+1's matmul stream (inputs are long since ready there,
            # so the PE never stalls on it) — only depth 3's tail runs after
            # the last main matmul.
            def emit_tail(d, xb):
                pt = ps_t.tile([128, 4 * B], bf16, name=f"pt{d}", tag="pt")
                for k in range(4):
                    nc.tensor.transpose(
                        pt[:, k * B : (k + 1) * B],
                        xb[:, k * 128 : (k + 1) * 128],
                        id_sb[:B, :B],
                    )
                xT = xpool.tile([128, 4 * B], bf16, name=f"xT{d}", tag=f"xT{d}")
                nc.vector.tensor_copy(xT[:], pt[:])

                c = COUNTS[d]
                nm = (c + 127) // 128
                pp = ps_p.tile([128, nm * B], f32, name=f"pp{d}", tag="pp")
                for m in range(nm):
                    ms = min(128, c - m * 128)
                    for k in range(4):
                        nc.tensor.matmul(
                            pp[:ms, m * B : m * B + B],
                            lhsT=wpt_sb[
                                :, k * L + LABEL_OFF[d] + m * 128 : k * L
                                + LABEL_OFF[d] + m * 128 + ms
                            ],
                            rhs=xT[:, k * B : (k + 1) * B],
                            start=(k == 0),
                            stop=(k == 3),
                        )
                for m in range(nm):
                    ms = min(128, c - m * 128)
                    po = spool.tile([128, B], f32, name=f"po{d}_{m}", tag="po")
                    nc.vector.tensor_copy(po[:ms, :], pp[:ms, m * B : m * B + B])
                    row0 = LABEL_OFF[d] + m * 128
                    nc.sync.dma_start(predT[row0 : row0 + ms, :], po[:ms, :])

            xb_tiles = []
            chunk_off = 0
            for d in range(3):
                nch = KCH[d]
                ps = ps_x.tile([B, H], f32, name=f"psx{d}", tag="psx")
                g0 = 0
                for gi, gl in enumerate(GROUPS[d]):
                    c0 = chunk_off + g0
                    # alternate the two HWDGE rings so the SDMA engines always
                    # have the next group's descriptors queued
                    ring = nc.sync if gi % 2 == 0 else nc.scalar
                    wt = wpool.tile([128, gl * WG], bf16, name="wt", tag="w")
                    ring.dma_start(wt[:], wg[:, c0 * WG : (c0 + gl) * WG])
                    for j in range(gl):
                        nc.tensor.matmul(
                            ps[:],
                            lhsT=wt[:, j * WG + H : (j + 1) * WG],
                            rhs=wt[:, j * WG : j * WG + H],
                            start=(g0 + j == 0),
                            stop=(g0 + j == nch - 1),
                        )
                    g0 += gl
                    if gi == 1 and d >= 1:
                        emit_tail(d - 1, xb_tiles[d - 1])
                chunk_off += nch
                # cast partial x to bf16 early (DVE runs concurrently with
                # the next depth's matmuls)
                xb = xpool.tile([B, H], bf16, name=f"xb{d}", tag=f"xb{d}")
                nc.vector.tensor_copy(xb[:], ps[:])
                xb_tiles.append(xb)

            emit_tail(2, xb_tiles[2])

    nc.finalize()
    return nc


def _prep_inputs(inputs):
    emb = np.asarray(inputs["label_aware_embedding"])
    W1s = [np.asarray(inputs[f"W1_{i + 1}"]) for i in range(3)]
    Wps = [np.asarray(inputs[f"Wp_{i + 1}"]) for i in range(3)]

    emb_bf = emb.astype(BF16)

    WG = H + B
    wg_all = np.empty((N_CORES, 128, NCH * WG), BF16)
    wgv = wg_all.reshape(N_CORES, 128, NCH, WG)
    off = 0
    for d in range(3):
        ch = KCH[d]
        W1T = np.ascontiguousarray(W1s[d].astype(BF16).T)  # [c*H, 512]
        wgv[:, :, off : off + ch, :H] = W1T.reshape(N_CORES, ch, 128, H).transpose(
            0, 2, 1, 3
        )
        ge = emb_bf[:, IDX[d], :]  # [B, c, H]
        GT = ge.transpose(1, 2, 0).reshape(-1, B)  # [c*H, 64]
        wgv[:, :, off : off + ch, H:] = GT.reshape(N_CORES, ch, 128, B).transpose(
            0, 2, 1, 3
        )
        off += ch

    WPT = np.concatenate([Wp.T for Wp in Wps], axis=1).astype(BF16)  # [512, 656]
    wpt_pack = np.ascontiguousarray(
        WPT.reshape(4, 128, L).transpose(1, 0, 2).reshape(128, 4 * L)
    )

    ident = np.eye(128, dtype=BF16)

    in_maps = []
    for c in range(N_CORES):
        in_maps.append(
            {
                "wg": wg_all[c],
                "wpt": wpt_pack,
                "ident": ident,
            }
        )
    return in_maps


LAST_RESULTS = None


def kernel(**inputs):
    global LAST_RESULTS
    if "nc" not in _CACHE:
        _CACHE["nc"] = _build_module()
    nc = _CACHE["nc"]
    in_maps = _prep_inputs(inputs)
    try:
        res = bass_utils.run_bass_kernel_spmd(
            nc, in_maps, core_ids=list(range(N_CORES))
        )
    except Exception:
        # transient NRT device errors have been observed; retry once
        res = bass_utils.run_bass_kernel_spmd(
            nc, in_maps, core_ids=list(range(N_CORES))
        )
    LAST_RESULTS = res

    # unshard: contraction was sharded, so the full predictor output is the
    # sum of the per-core partials; add the bias once at the end.
    total = np.zeros((L, B), np.float64)
    for c in range(N_CORES):
        total += res.results[c]["predT"]
    bias = np.concatenate([np.asarray(inputs[f"bp_{i + 1}"]) for i in range(3)])
    total += bias.astype(np.float64)[:, None]
    out = np.empty((B, L), np.float32)
    out[:, ORDER] = total.T.astype(np.float32)
    return out
